# revision 8
# baseline (speedup 1.0000x reference)
"""Trainium2 Bass kernel for CharPredictorMultirateFFN.

Model: emb = emb_table[tokens]; conv = relu(causal_conv1d(emb, K=16) + b);
logits = cat(emb, conv) @ lin_w.T + lin_b; out = softmax(logits).

Key algebraic restructure (tokens take only V=256 values):
  conv[s, h] = sum_k U[tok[s-15+k], k, h]   with U[v,k,h] = sum_e emb[v,e] conv_w[h,e,k]
so the conv becomes 16 shifted one-hot matmuls with contract dim 256 (half the
FLOPs of the direct E=512 conv) and the one-hot operand is exact in fp8.
The emb half of the final linear folds into P1 = emb_table @ lin_w[:, :E].T
(one-hot matmul, [256,256]), removing the embedding gather entirely.

fp8 DoubleRow: U and the one-hot are stored fp8-e4m3; each PE matmul runs in
MatmulPerfMode.DoubleRow with the two 128-row streams carrying the lo/hi
halves of the 256-wide one-hot, so one instruction contracts a full tap
(256) in the time fp16 contracts 128 -> conv runs at 157 TF/s/core, 2x the
fp16 peak. The one-hot is exact in fp8; only U is quantized (~2.2% RMS per
entry, 16-term sums -> ~1.5e-2 rel_l2 on the softmax output). Stage 3
(R @ W2T + gathered P1 rows, softmax) stays fp16/fp32.

Sharding: data-parallel over batch - 4 sequences per core on 8 cores, all
tables replicated, no collectives.

biases are folded host-side: conv_b into U[:, K-1, :] (tap k=15 is always
valid for every output position), lin_b into P1 rows (shift-0 one-hot always
valid), so the device kernel has no bias adds.
"""

import numpy as np
import ml_dtypes

B, S, V, E, H, K = 32, 2048, 256, 512, 1024, 16
NCORES = 8
SEQ_PER_CORE = B // NCORES            # 4
PAD = K - 1                           # 15
SPAD = S + PAD                        # 2063
H8 = H // 128                         # 8
NTT = S // 512                        # 4 token-tiles of 512 per sequence
F16 = np.float16
F8 = ml_dtypes.float8_e4m3

TRACE = False          # set True (e.g. from test.py) to capture NTFF profile
LAST_RESULT = None     # BassKernelResults of the most recent run

_NC_CACHE = {}


def _build_nc(seq_per_core=SEQ_PER_CORE, ntt=NTT):
    """Build the Bass module (SPMD, identical program on every core)."""
    from contextlib import ExitStack
    import concourse.bacc as bacc
    import concourse.tile as tile
    import concourse.mybir as mybir

    f32 = mybir.dt.float32
    f16 = mybir.dt.float16
    f8 = mybir.dt.float8e4
    DR = mybir.MatmulPerfMode.DoubleRow
    AF = mybir.ActivationFunctionType
    toks = seq_per_core * ntt * 512

    nc = bacc.Bacc("TRN2", target_bir_lowering=False, debug=False,
                   num_devices=NCORES)

    # u layout: [part, k, hc, vh, 128] so lhsT for (k, h-chunk) is the
    # [128, 2, 128] DoubleRow stationary AP and per-k slabs for the first
    # conv group are contiguous and can land early via sliced DMAs.
    oh_d = nc.dram_tensor("oh", [128, 2, seq_per_core, SPAD], f8,
                          kind="ExternalInput").ap()
    u_d = nc.dram_tensor("u", [128, K, H8, 2, 128], f8,
                         kind="ExternalInput").ap()
    w2_d = nc.dram_tensor("w2", [128, H8, V], f16,
                          kind="ExternalInput").ap()
    # host-gathered P1[tok] rows (emb half of the linear; lin_b folded in):
    # [tile, p, m, v] = row tile*512 + m*128 + p, so each token-tile is one
    # contiguous [128, 4, V] DMA.
    pe_d = nc.dram_tensor("pe", [seq_per_core * ntt, 128, 4, V], f32,
                          kind="ExternalInput").ap()
    out_d = nc.dram_tensor("out", [toks, V], f32, kind="ExternalOutput").ap()

    with tile.TileContext(nc) as tc, ExitStack() as ctx:
        consts = ctx.enter_context(tc.tile_pool(name="consts", bufs=1))
        u_t = consts.tile([128, K, H8, 2, 128], f8, name="u_t")
        oh_t = consts.tile([128, 2, seq_per_core, SPAD], f8, name="oh_t")
        w2_t = consts.tile([128, H8, V], f16, name="w2_t")
        # staggered loads ordered along the kernel's critical path: the
        # first conv group consumes u[:, k, 0:4] in k order on oh[b=0,
        # cols<528], so stream those slabs first in small chunks. The oh
        # head chunk goes out on the Activation HWDGE queue so its issue
        # doesn't serialize behind the u chunks on SP.
        nc.scalar.dma_start(oh_t[:, :, 0, 0:528], oh_d[:, :, 0, 0:528])
        qs = [nc.sync, nc.scalar, nc.gpsimd]
        for kq in range(K):
            qs[kq % 3].dma_start(u_t[:, kq:kq + 1, 0:4],
                                 u_d[:, kq:kq + 1, 0:4])
        KQ = 4
        for kq in range(0, K, KQ):
            nc.sync.dma_start(u_t[:, kq:kq + KQ, 4:8],
                              u_d[:, kq:kq + KQ, 4:8])
        nc.scalar.dma_start(oh_t[:, :, 0, 528:SPAD], oh_d[:, :, 0, 528:SPAD])
        for b in range(1, seq_per_core):
            nc.sync.dma_start(oh_t[:, :, b, :], oh_d[:, :, b, :])
        nc.sync.dma_start(w2_t[:], w2_d[:])

        pe_pool = ctx.enter_context(tc.tile_pool(name="pep", bufs=3))
        r_pool = ctx.enter_context(tc.tile_pool(name="rp", bufs=3))
        cps = ctx.enter_context(tc.tile_pool(name="cps", bufs=6, space="PSUM"))

        # PE warm-up: the HAM clock gate holds the PE at 1.2 GHz until it
        # has been busy ~3.4us. Run throwaway matmuls while the input DMAs
        # are in flight so the real stream starts at 2.4 GHz with no cold
        # ramp. Operands are deliberately uninitialized (no memset: that
        # would wait on the Vector engine's preamble and delay the PE);
        # the results land in a PSUM bank that the first real accumulation
        # group resets via start=True.
        wlhs = consts.tile([128, 128], f16, name="wlhs")
        wrhs = consts.tile([128, 512], f16, name="wrhs")
        nc.gpsimd.memset(wlhs[:], 0)
        nc.vector.memset(wrhs[:], 0)
        wp = cps.tile([128, 512], f32, name="warmps", tag="cp")
        for _ in range(8):
            nc.tensor.matmul(wp[:], wlhs[:], wrhs[:], start=True, stop=True)
        lps = ctx.enter_context(tc.tile_pool(name="lps", bufs=2, space="PSUM"))
        sm_pool = ctx.enter_context(tc.tile_pool(name="smp", bufs=4))
        out_pool = ctx.enter_context(tc.tile_pool(name="outp", bufs=4))

        def conv_emit(b, tt):
            """Conv for 512 tokens -> relu -> fp16 R tile [128, H8, 512]."""
            pe_t = pe_pool.tile([128, 4, V], f32, name="pe_t", tag="pe")
            nc.sync.dma_start(pe_t[:], pe_d[b * ntt + tt])
            rt = r_pool.tile([128, H8, 512], f16, name="rt", tag="rt")
            col0 = tt * 512
            for g in range(2):           # 4 PSUM banks per group of 4 h-chunks
                ps = [cps.tile([128, 512], f32, name=f"cp{i}", tag="cp")
                      for i in range(4)]
                for k in range(K):
                    rhs = oh_t[:, :, b, col0 + k: col0 + k + 512]
                    for i in range(4):
                        nc.tensor.matmul(
                            ps[i][:], u_t[:, k, g * 4 + i], rhs,
                            start=(k == 0), stop=(k == K - 1),
                            perf_mode=DR)
                for i in range(4):
                    nc.scalar.activation(rt[:, g * 4 + i, :], ps[i][:], AF.Relu)
            return rt, pe_t

        def stage3_emit(b, tt, rt, pe_t):
            """logits = R@W2T (PE) + gathered P1 rows (DVE add), softmax."""
            for m in range(4):
                psl = lps.tile([128, V], f32, name="psl", tag="psl")
                for h8 in range(H8):
                    nc.tensor.matmul(
                        psl[:], rt[:, h8, m * 128:(m + 1) * 128],
                        w2_t[:, h8, :],
                        start=(h8 == 0), stop=(h8 == H8 - 1))
                li = sm_pool.tile([128, V], f32, name="li", tag="li")
                nc.vector.tensor_add(li[:], psl[:], pe_t[:, m, :])
                et = sm_pool.tile([128, V], f32, name="et", tag="et")
                ssum = sm_pool.tile([128, 1], f32, name="ssum", tag="ssum")
                nc.scalar.activation(et[:], li[:], AF.Exp, accum_out=ssum[:])
                rec = sm_pool.tile([128, 1], f32, name="rec", tag="rec")
                nc.vector.reciprocal(rec[:], ssum[:])
                ot = out_pool.tile([128, V], f32, name="ot", tag="ot")
                nc.vector.tensor_scalar_mul(ot[:], et[:], rec[:])
                row0 = (b * ntt + tt) * 512 + m * 128
                nc.sync.dma_start(out_d[row0:row0 + 128, :], ot[:])

        # software pipeline: stage3 of tile i runs on the PE while ACT is
        # still free to relu tile i+1's PSUM -> no PE stall on the relu.
        tiles = [(b, tt) for b in range(seq_per_core) for tt in range(ntt)]
        prev = None
        for (b, tt) in tiles:
            rt, pe_t = conv_emit(b, tt)
            if prev is not None:
                stage3_emit(*prev)
            prev = (b, tt, rt, pe_t)
        stage3_emit(*prev)

    nc.compile()
    return nc


def _get_nc():
    if "full" not in _NC_CACHE:
        _NC_CACHE["full"] = _build_nc()
    return _NC_CACHE["full"]


def _pack_tables(emb_table, conv_w, conv_b, lin_w, lin_b):
    """Host-side table precompute + fp8/fp16 packing (a weight repack)."""
    emb_table = np.asarray(emb_table, np.float32)
    conv_w = np.asarray(conv_w, np.float32)
    lin_w = np.asarray(lin_w, np.float32)
    # U[v,k,h] = sum_e emb[v,e] * conv_w[h,e,k]
    U = (emb_table @ conv_w.transpose(1, 0, 2).reshape(E, H * K))
    U = U.reshape(V, H, K).transpose(0, 2, 1).copy()       # [V, K, H]
    U[:, K - 1, :] += np.asarray(conv_b, np.float32)
    P1 = emb_table @ lin_w[:, :E].T + np.asarray(lin_b, np.float32)[None, :]
    W2T = lin_w[:, E:].T.copy()                            # [H, V]

    # [128, k, hc, vh, 128]: u_p[p, k, hc, vh, m] = U[vh*128+p, k, hc*128+m]
    u_p = (U.reshape(2, 128, K, H8, 128)
           .transpose(1, 2, 3, 0, 4)).astype(F8)
    w2_p = W2T.reshape(H8, 128, V).transpose(1, 0, 2).astype(F16)
    return np.ascontiguousarray(u_p), np.ascontiguousarray(w2_p), P1


def _onehot(tokens):
    """[128, 2, B, SPAD] fp8, left-padded with 15 zero columns per sequence."""
    tok = np.asarray(tokens).astype(np.int64)
    oh = np.zeros((128, 2, B, SPAD), F8)
    t = tok.ravel()
    b_idx = np.repeat(np.arange(B), S)
    col = np.tile(np.arange(S), B) + PAD
    oh[t % 128, t // 128, b_idx, col] = 1
    return oh


def kernel(input_sequence, emb_table, conv_w, conv_b, lin_w, lin_b):
    global LAST_RESULT
    import os
    if not TRACE:
        # the container's antenv lacks the axon NTFF hook; make sure an
        # ambient BASS_TRACE can't route us into that import path
        os.environ["BASS_NEVER_TRACE"] = "1"
    else:
        os.environ.pop("BASS_NEVER_TRACE", None)
    from concourse.bass_utils import run_bass_kernel_spmd

    u_p, w2_p, P1 = _pack_tables(emb_table, conv_w, conv_b, lin_w, lin_b)
    oh_full = _onehot(input_sequence)
    # emb-side logits: gather P1 rows per token, packed per 512-token tile
    # as [tile, p, m, v] with token row = tile*512 + m*128 + p
    tok = np.asarray(input_sequence).astype(np.int64)
    pe_all = P1[tok].astype(np.float32)                      # [B, S, V]
    pe_all = (pe_all.reshape(B * S // 512, 4, 128, V)
              .transpose(0, 2, 1, 3))                  # [tiles, 128, 4, V]

    ntt_core = SEQ_PER_CORE * NTT
    in_maps = []
    for c in range(NCORES):
        in_maps.append({
            "oh": np.ascontiguousarray(
                oh_full[:, :, c * SEQ_PER_CORE:(c + 1) * SEQ_PER_CORE, :]),
            "u": u_p, "w2": w2_p,
            "pe": np.ascontiguousarray(
                pe_all[c * ntt_core:(c + 1) * ntt_core]),
        })

    nc = _get_nc()
    res = run_bass_kernel_spmd(nc, in_maps, core_ids=list(range(NCORES)),
                               trace=TRACE)
    LAST_RESULT = res
    outs = [res.results[c]["out"] for c in range(NCORES)]   # [8192, 256] each
    full = np.concatenate(outs, axis=0).reshape(B, S, V)
    return np.ascontiguousarray(full.astype(np.float32))


# revision 9
# speedup vs baseline: 1.0005x; 1.0005x over previous
"""Trainium2 Bass kernel for CharPredictorMultirateFFN.

Model: emb = emb_table[tokens]; conv = relu(causal_conv1d(emb, K=16) + b);
logits = cat(emb, conv) @ lin_w.T + lin_b; out = softmax(logits).

Key algebraic restructure (tokens take only V=256 values):
  conv[s, h] = sum_k U[tok[s-15+k], k, h]   with U[v,k,h] = sum_e emb[v,e] conv_w[h,e,k]
so the conv becomes 16 shifted one-hot matmuls with contract dim 256 (half the
FLOPs of the direct E=512 conv) and the one-hot operand is exact in fp8.
The emb half of the final linear folds into P1 = emb_table @ lin_w[:, :E].T
(one-hot matmul, [256,256]), removing the embedding gather entirely.

fp8 DoubleRow: U and the one-hot are stored fp8-e4m3; each PE matmul runs in
MatmulPerfMode.DoubleRow with the two 128-row streams carrying the lo/hi
halves of the 256-wide one-hot, so one instruction contracts a full tap
(256) in the time fp16 contracts 128 -> conv runs at 157 TF/s/core, 2x the
fp16 peak. The one-hot is exact in fp8; only U is quantized (~2.2% RMS per
entry, 16-term sums -> ~1.5e-2 rel_l2 on the softmax output). Stage 3
(R @ W2T + gathered P1 rows, softmax) stays fp16/fp32.

Sharding: data-parallel over batch - 4 sequences per core on 8 cores, all
tables replicated, no collectives.

biases are folded host-side: conv_b into U[:, K-1, :] (tap k=15 is always
valid for every output position), lin_b into P1 rows (shift-0 one-hot always
valid), so the device kernel has no bias adds.
"""

import numpy as np
import ml_dtypes

B, S, V, E, H, K = 32, 2048, 256, 512, 1024, 16
NCORES = 8
SEQ_PER_CORE = B // NCORES            # 4
PAD = K - 1                           # 15
SPAD = S + PAD                        # 2063
H8 = H // 128                         # 8
NTT = S // 512                        # 4 token-tiles of 512 per sequence
F16 = np.float16
F8 = ml_dtypes.float8_e4m3

TRACE = False          # set True (e.g. from test.py) to capture NTFF profile
LAST_RESULT = None     # BassKernelResults of the most recent run

_NC_CACHE = {}


def _build_nc(seq_per_core=SEQ_PER_CORE, ntt=NTT):
    """Build the Bass module (SPMD, identical program on every core)."""
    from contextlib import ExitStack
    import concourse.bacc as bacc
    import concourse.tile as tile
    import concourse.mybir as mybir

    f32 = mybir.dt.float32
    f16 = mybir.dt.float16
    f8 = mybir.dt.float8e4
    DR = mybir.MatmulPerfMode.DoubleRow
    AF = mybir.ActivationFunctionType
    toks = seq_per_core * ntt * 512

    nc = bacc.Bacc("TRN2", target_bir_lowering=False, debug=False,
                   num_devices=NCORES)

    # u layout: [part, k, hc, vh, 128] so lhsT for (k, h-chunk) is the
    # [128, 2, 128] DoubleRow stationary AP and per-k slabs for the first
    # conv group are contiguous and can land early via sliced DMAs.
    oh_d = nc.dram_tensor("oh", [128, 2, seq_per_core, SPAD], f8,
                          kind="ExternalInput").ap()
    u_d = nc.dram_tensor("u", [128, K, H8, 2, 128], f8,
                         kind="ExternalInput").ap()
    w2_d = nc.dram_tensor("w2", [128, H8, V], f16,
                          kind="ExternalInput").ap()
    # host-gathered P1[tok] rows (emb half of the linear; lin_b folded in):
    # [tile, p, m, v] = row tile*512 + m*128 + p, so each token-tile is one
    # contiguous [128, 4, V] DMA.
    pe_d = nc.dram_tensor("pe", [seq_per_core * ntt, 128, 4, V], f32,
                          kind="ExternalInput").ap()
    out_d = nc.dram_tensor("out", [toks, V], f32, kind="ExternalOutput").ap()

    with tile.TileContext(nc) as tc, ExitStack() as ctx:
        consts = ctx.enter_context(tc.tile_pool(name="consts", bufs=1))
        u_t = consts.tile([128, K, H8, 2, 128], f8, name="u_t")
        oh_t = consts.tile([128, 2, seq_per_core, SPAD], f8, name="oh_t")
        w2_t = consts.tile([128, H8, V], f16, name="w2_t")
        # staggered loads ordered along the kernel's critical path: the
        # first conv group consumes u[:, k, 0:4] in k order on oh[b=0,
        # cols<528], so stream those slabs first in small chunks. The oh
        # head chunk goes out on the Activation HWDGE queue so its issue
        # doesn't serialize behind the u chunks on SP.
        nc.scalar.dma_start(oh_t[:, :, 0, 0:528], oh_d[:, :, 0, 0:528])
        qs = [nc.sync, nc.scalar, nc.gpsimd]
        for kq in range(K):
            qs[kq % 3].dma_start(u_t[:, kq:kq + 1, 0:4],
                                 u_d[:, kq:kq + 1, 0:4])
        KQ = 4
        for kq in range(0, K, KQ):
            nc.sync.dma_start(u_t[:, kq:kq + KQ, 4:8],
                              u_d[:, kq:kq + KQ, 4:8])
        nc.scalar.dma_start(oh_t[:, :, 0, 528:SPAD], oh_d[:, :, 0, 528:SPAD])
        for b in range(1, seq_per_core):
            nc.sync.dma_start(oh_t[:, :, b, :], oh_d[:, :, b, :])
        nc.sync.dma_start(w2_t[:], w2_d[:])

        pe_pool = ctx.enter_context(tc.tile_pool(name="pep", bufs=3))
        r_pool = ctx.enter_context(tc.tile_pool(name="rp", bufs=3))
        cps = ctx.enter_context(tc.tile_pool(name="cps", bufs=6, space="PSUM"))

        # PE warm-up: the HAM clock gate holds the PE at 1.2 GHz until it
        # has been busy ~3.4us. Run throwaway matmuls while the input DMAs
        # are in flight so the real stream starts at 2.4 GHz with no cold
        # ramp. Operands are deliberately uninitialized (no memset: that
        # would wait on the Vector engine's preamble and delay the PE);
        # the results land in a PSUM bank that the first real accumulation
        # group resets via start=True.
        wlhs = consts.tile([128, 128], f16, name="wlhs")
        wrhs = consts.tile([128, 512], f16, name="wrhs")
        nc.gpsimd.memset(wlhs[:], 0)
        nc.gpsimd.memset(wrhs[:], 0)
        wp = cps.tile([128, 512], f32, name="warmps", tag="cp")
        for _ in range(10):
            nc.tensor.matmul(wp[:], wlhs[:], wrhs[:], start=True, stop=True)
        lps = ctx.enter_context(tc.tile_pool(name="lps", bufs=2, space="PSUM"))
        sm_pool = ctx.enter_context(tc.tile_pool(name="smp", bufs=4))
        out_pool = ctx.enter_context(tc.tile_pool(name="outp", bufs=4))

        def conv_emit(b, tt):
            """Conv for 512 tokens -> relu -> fp16 R tile [128, H8, 512]."""
            pe_t = pe_pool.tile([128, 4, V], f32, name="pe_t", tag="pe")
            nc.sync.dma_start(pe_t[:], pe_d[b * ntt + tt])
            rt = r_pool.tile([128, H8, 512], f16, name="rt", tag="rt")
            col0 = tt * 512
            for g in range(2):           # 4 PSUM banks per group of 4 h-chunks
                ps = [cps.tile([128, 512], f32, name=f"cp{i}", tag="cp")
                      for i in range(4)]
                for k in range(K):
                    rhs = oh_t[:, :, b, col0 + k: col0 + k + 512]
                    for i in range(4):
                        nc.tensor.matmul(
                            ps[i][:], u_t[:, k, g * 4 + i], rhs,
                            start=(k == 0), stop=(k == K - 1),
                            perf_mode=DR)
                for i in range(4):
                    nc.scalar.activation(rt[:, g * 4 + i, :], ps[i][:], AF.Relu)
            return rt, pe_t

        def stage3_emit(b, tt, rt, pe_t):
            """logits = R@W2T (PE) + gathered P1 rows (DVE add), softmax."""
            for m in range(4):
                psl = lps.tile([128, V], f32, name="psl", tag="psl")
                for h8 in range(H8):
                    nc.tensor.matmul(
                        psl[:], rt[:, h8, m * 128:(m + 1) * 128],
                        w2_t[:, h8, :],
                        start=(h8 == 0), stop=(h8 == H8 - 1))
                li = sm_pool.tile([128, V], f32, name="li", tag="li")
                nc.vector.tensor_add(li[:], psl[:], pe_t[:, m, :])
                et = sm_pool.tile([128, V], f32, name="et", tag="et")
                ssum = sm_pool.tile([128, 1], f32, name="ssum", tag="ssum")
                nc.scalar.activation(et[:], li[:], AF.Exp, accum_out=ssum[:])
                rec = sm_pool.tile([128, 1], f32, name="rec", tag="rec")
                nc.vector.reciprocal(rec[:], ssum[:])
                ot = out_pool.tile([128, V], f32, name="ot", tag="ot")
                nc.vector.tensor_scalar_mul(ot[:], et[:], rec[:])
                row0 = (b * ntt + tt) * 512 + m * 128
                nc.sync.dma_start(out_d[row0:row0 + 128, :], ot[:])

        # software pipeline: stage3 of tile i runs on the PE while ACT is
        # still free to relu tile i+1's PSUM -> no PE stall on the relu.
        tiles = [(b, tt) for b in range(seq_per_core) for tt in range(ntt)]
        prev = None
        for (b, tt) in tiles:
            rt, pe_t = conv_emit(b, tt)
            if prev is not None:
                stage3_emit(*prev)
            prev = (b, tt, rt, pe_t)
        stage3_emit(*prev)

    nc.compile()
    return nc


def _get_nc():
    if "full" not in _NC_CACHE:
        _NC_CACHE["full"] = _build_nc()
    return _NC_CACHE["full"]


def _pack_tables(emb_table, conv_w, conv_b, lin_w, lin_b):
    """Host-side table precompute + fp8/fp16 packing (a weight repack)."""
    emb_table = np.asarray(emb_table, np.float32)
    conv_w = np.asarray(conv_w, np.float32)
    lin_w = np.asarray(lin_w, np.float32)
    # U[v,k,h] = sum_e emb[v,e] * conv_w[h,e,k]
    U = (emb_table @ conv_w.transpose(1, 0, 2).reshape(E, H * K))
    U = U.reshape(V, H, K).transpose(0, 2, 1).copy()       # [V, K, H]
    U[:, K - 1, :] += np.asarray(conv_b, np.float32)
    P1 = emb_table @ lin_w[:, :E].T + np.asarray(lin_b, np.float32)[None, :]
    W2T = lin_w[:, E:].T.copy()                            # [H, V]

    # [128, k, hc, vh, 128]: u_p[p, k, hc, vh, m] = U[vh*128+p, k, hc*128+m]
    u_p = (U.reshape(2, 128, K, H8, 128)
           .transpose(1, 2, 3, 0, 4)).astype(F8)
    w2_p = W2T.reshape(H8, 128, V).transpose(1, 0, 2).astype(F16)
    return np.ascontiguousarray(u_p), np.ascontiguousarray(w2_p), P1


def _onehot(tokens):
    """[128, 2, B, SPAD] fp8, left-padded with 15 zero columns per sequence."""
    tok = np.asarray(tokens).astype(np.int64)
    oh = np.zeros((128, 2, B, SPAD), F8)
    t = tok.ravel()
    b_idx = np.repeat(np.arange(B), S)
    col = np.tile(np.arange(S), B) + PAD
    oh[t % 128, t // 128, b_idx, col] = 1
    return oh


def kernel(input_sequence, emb_table, conv_w, conv_b, lin_w, lin_b):
    global LAST_RESULT
    import os
    if not TRACE:
        # the container's antenv lacks the axon NTFF hook; make sure an
        # ambient BASS_TRACE can't route us into that import path
        os.environ["BASS_NEVER_TRACE"] = "1"
    else:
        os.environ.pop("BASS_NEVER_TRACE", None)
    from concourse.bass_utils import run_bass_kernel_spmd

    u_p, w2_p, P1 = _pack_tables(emb_table, conv_w, conv_b, lin_w, lin_b)
    oh_full = _onehot(input_sequence)
    # emb-side logits: gather P1 rows per token, packed per 512-token tile
    # as [tile, p, m, v] with token row = tile*512 + m*128 + p
    tok = np.asarray(input_sequence).astype(np.int64)
    pe_all = P1[tok].astype(np.float32)                      # [B, S, V]
    pe_all = (pe_all.reshape(B * S // 512, 4, 128, V)
              .transpose(0, 2, 1, 3))                  # [tiles, 128, 4, V]

    ntt_core = SEQ_PER_CORE * NTT
    in_maps = []
    for c in range(NCORES):
        in_maps.append({
            "oh": np.ascontiguousarray(
                oh_full[:, :, c * SEQ_PER_CORE:(c + 1) * SEQ_PER_CORE, :]),
            "u": u_p, "w2": w2_p,
            "pe": np.ascontiguousarray(
                pe_all[c * ntt_core:(c + 1) * ntt_core]),
        })

    nc = _get_nc()
    res = run_bass_kernel_spmd(nc, in_maps, core_ids=list(range(NCORES)),
                               trace=TRACE)
    LAST_RESULT = res
    outs = [res.results[c]["out"] for c in range(NCORES)]   # [8192, 256] each
    full = np.concatenate(outs, axis=0).reshape(B, S, V)
    return np.ascontiguousarray(full.astype(np.float32))


# revision 10
# speedup vs baseline: 1.0062x; 1.0057x over previous
"""Trainium2 Bass kernel for CharPredictorMultirateFFN.

Model: emb = emb_table[tokens]; conv = relu(causal_conv1d(emb, K=16) + b);
logits = cat(emb, conv) @ lin_w.T + lin_b; out = softmax(logits).

Key algebraic restructure (tokens take only V=256 values):
  conv[s, h] = sum_k U[tok[s-15+k], k, h]   with U[v,k,h] = sum_e emb[v,e] conv_w[h,e,k]
so the conv becomes 16 shifted one-hot matmuls with contract dim 256 (half the
FLOPs of the direct E=512 conv) and the one-hot operand is exact in fp8.
The emb half of the final linear folds into P1 = emb_table @ lin_w[:, :E].T
(one-hot matmul, [256,256]), removing the embedding gather entirely.

fp8 DoubleRow: U and the one-hot are stored fp8-e4m3; each PE matmul runs in
MatmulPerfMode.DoubleRow with the two 128-row streams carrying the lo/hi
halves of the 256-wide one-hot, so one instruction contracts a full tap
(256) in the time fp16 contracts 128 -> conv runs at 157 TF/s/core, 2x the
fp16 peak. The one-hot is exact in fp8; only U is quantized (~2.2% RMS per
entry, 16-term sums -> ~1.5e-2 rel_l2 on the softmax output). Stage 3
(R @ W2T + gathered P1 rows, softmax) stays fp16/fp32.

Sharding: data-parallel over batch - 4 sequences per core on 8 cores, all
tables replicated, no collectives.

biases are folded host-side: conv_b into U[:, K-1, :] (tap k=15 is always
valid for every output position), lin_b into P1 rows (shift-0 one-hot always
valid), so the device kernel has no bias adds.
"""

import numpy as np
import ml_dtypes

B, S, V, E, H, K = 32, 2048, 256, 512, 1024, 16
NCORES = 8
SEQ_PER_CORE = B // NCORES            # 4
PAD = K - 1                           # 15
SPAD = S + PAD                        # 2063
H8 = H // 128                         # 8
NTT = S // 512                        # 4 token-tiles of 512 per sequence
F16 = np.float16
F8 = ml_dtypes.float8_e4m3

TRACE = False          # set True (e.g. from test.py) to capture NTFF profile
LAST_RESULT = None     # BassKernelResults of the most recent run

_NC_CACHE = {}


def _build_nc(seq_per_core=SEQ_PER_CORE, ntt=NTT):
    """Build the Bass module (SPMD, identical program on every core)."""
    from contextlib import ExitStack
    import concourse.bacc as bacc
    import concourse.tile as tile
    import concourse.mybir as mybir

    f32 = mybir.dt.float32
    f16 = mybir.dt.float16
    f8 = mybir.dt.float8e4
    DR = mybir.MatmulPerfMode.DoubleRow
    AF = mybir.ActivationFunctionType
    toks = seq_per_core * ntt * 512

    nc = bacc.Bacc("TRN2", target_bir_lowering=False, debug=False,
                   num_devices=NCORES)

    # u layout: [part, k, hc, vh, 128] so lhsT for (k, h-chunk) is the
    # [128, 2, 128] DoubleRow stationary AP and per-k slabs for the first
    # conv group are contiguous and can land early via sliced DMAs.
    oh_d = nc.dram_tensor("oh", [128, 2, seq_per_core, SPAD], f8,
                          kind="ExternalInput").ap()
    u_d = nc.dram_tensor("u", [128, K, H8, 2, 128], f8,
                         kind="ExternalInput").ap()
    w2_d = nc.dram_tensor("w2", [128, H8, V], f16,
                          kind="ExternalInput").ap()
    # host-gathered P1[tok] rows (emb half of the linear; lin_b folded in):
    # [tile, p, m, v] = row tile*512 + m*128 + p, so each token-tile is one
    # contiguous [128, 4, V] DMA.
    pe_d = nc.dram_tensor("pe", [seq_per_core * ntt, 128, 4, V], f32,
                          kind="ExternalInput").ap()
    out_d = nc.dram_tensor("out", [toks, V], f32, kind="ExternalOutput").ap()

    with tile.TileContext(nc) as tc, ExitStack() as ctx:
        consts = ctx.enter_context(tc.tile_pool(name="consts", bufs=1))
        u_t = consts.tile([128, K, H8, 2, 128], f8, name="u_t")
        oh_t = consts.tile([128, 2, seq_per_core, SPAD], f8, name="oh_t")
        w2_t = consts.tile([128, H8, V], f16, name="w2_t")
        # staggered loads ordered along the kernel's critical path: the
        # first conv group consumes u[:, k, 0:4] in k order on oh[b=0,
        # cols<528], so stream those slabs first in small chunks. The oh
        # head chunk goes out on the Activation HWDGE queue so its issue
        # doesn't serialize behind the u chunks on SP.
        nc.scalar.dma_start(oh_t[:, :, 0, 0:528], oh_d[:, :, 0, 0:528])
        for kq in range(K):
            eng = nc.sync if kq % 2 == 0 else nc.scalar
            eng.dma_start(u_t[:, kq:kq + 1, 0:4], u_d[:, kq:kq + 1, 0:4])
        KQ = 4
        for kq in range(0, K, KQ):
            nc.sync.dma_start(u_t[:, kq:kq + KQ, 4:8],
                              u_d[:, kq:kq + KQ, 4:8])
        nc.sync.dma_start(oh_t[:, :, 0, 528:SPAD], oh_d[:, :, 0, 528:SPAD])
        for b in range(1, seq_per_core):
            nc.sync.dma_start(oh_t[:, :, b, :], oh_d[:, :, b, :])
        nc.sync.dma_start(w2_t[:], w2_d[:])

        pe_pool = ctx.enter_context(tc.tile_pool(name="pep", bufs=3))
        r_pool = ctx.enter_context(tc.tile_pool(name="rp", bufs=3))
        cps = ctx.enter_context(tc.tile_pool(name="cps", bufs=6, space="PSUM"))

        # PE warm-up: the HAM clock gate holds the PE at 1.2 GHz until it
        # has been busy ~3.4us. Run throwaway matmuls while the input DMAs
        # are in flight so the real stream starts at 2.4 GHz with no cold
        # ramp. Operands are deliberately uninitialized (no memset: that
        # would wait on the Vector engine's preamble and delay the PE);
        # the results land in a PSUM bank that the first real accumulation
        # group resets via start=True.
        wlhs = consts.tile([128, 128], f16, name="wlhs")
        wrhs = consts.tile([128, 512], f16, name="wrhs")
        nc.gpsimd.memset(wlhs[:], 0)
        nc.gpsimd.memset(wrhs[:], 0)
        wp = cps.tile([128, 512], f32, name="warmps", tag="cp")
        for _ in range(10):
            nc.tensor.matmul(wp[:], wlhs[:], wrhs[:], start=True, stop=True)
        lps = ctx.enter_context(tc.tile_pool(name="lps", bufs=2, space="PSUM"))
        sm_pool = ctx.enter_context(tc.tile_pool(name="smp", bufs=4))
        out_pool = ctx.enter_context(tc.tile_pool(name="outp", bufs=4))

        def conv_emit(b, tt):
            """Conv for 512 tokens -> relu -> fp16 R tile [128, H8, 512]."""
            pe_t = pe_pool.tile([128, 4, V], f32, name="pe_t", tag="pe")
            nc.sync.dma_start(pe_t[:], pe_d[b * ntt + tt])
            rt = r_pool.tile([128, H8, 512], f16, name="rt", tag="rt")
            col0 = tt * 512
            for g in range(2):           # 4 PSUM banks per group of 4 h-chunks
                ps = [cps.tile([128, 512], f32, name=f"cp{i}", tag="cp")
                      for i in range(4)]
                for k in range(K):
                    rhs = oh_t[:, :, b, col0 + k: col0 + k + 512]
                    for i in range(4):
                        nc.tensor.matmul(
                            ps[i][:], u_t[:, k, g * 4 + i], rhs,
                            start=(k == 0), stop=(k == K - 1),
                            perf_mode=DR)
                for i in range(4):
                    nc.scalar.activation(rt[:, g * 4 + i, :], ps[i][:], AF.Relu)
            return rt, pe_t

        def stage3_emit(b, tt, rt, pe_t):
            """logits = R@W2T (PE) + gathered P1 rows (DVE add), softmax."""
            for m in range(4):
                psl = lps.tile([128, V], f32, name="psl", tag="psl")
                for h8 in range(H8):
                    nc.tensor.matmul(
                        psl[:], rt[:, h8, m * 128:(m + 1) * 128],
                        w2_t[:, h8, :],
                        start=(h8 == 0), stop=(h8 == H8 - 1))
                li = sm_pool.tile([128, V], f32, name="li", tag="li")
                nc.vector.tensor_add(li[:], psl[:], pe_t[:, m, :])
                et = sm_pool.tile([128, V], f32, name="et", tag="et")
                ssum = sm_pool.tile([128, 1], f32, name="ssum", tag="ssum")
                nc.scalar.activation(et[:], li[:], AF.Exp, accum_out=ssum[:])
                rec = sm_pool.tile([128, 1], f32, name="rec", tag="rec")
                nc.vector.reciprocal(rec[:], ssum[:])
                ot = out_pool.tile([128, V], f32, name="ot", tag="ot")
                nc.vector.tensor_scalar_mul(ot[:], et[:], rec[:])
                row0 = (b * ntt + tt) * 512 + m * 128
                nc.sync.dma_start(out_d[row0:row0 + 128, :], ot[:])

        # software pipeline: stage3 of tile i runs on the PE while ACT is
        # still free to relu tile i+1's PSUM -> no PE stall on the relu.
        tiles = [(b, tt) for b in range(seq_per_core) for tt in range(ntt)]
        prev = None
        for (b, tt) in tiles:
            rt, pe_t = conv_emit(b, tt)
            if prev is not None:
                stage3_emit(*prev)
            prev = (b, tt, rt, pe_t)
        stage3_emit(*prev)

    nc.compile()
    return nc


def _get_nc():
    if "full" not in _NC_CACHE:
        _NC_CACHE["full"] = _build_nc()
    return _NC_CACHE["full"]


def _pack_tables(emb_table, conv_w, conv_b, lin_w, lin_b):
    """Host-side table precompute + fp8/fp16 packing (a weight repack)."""
    emb_table = np.asarray(emb_table, np.float32)
    conv_w = np.asarray(conv_w, np.float32)
    lin_w = np.asarray(lin_w, np.float32)
    # U[v,k,h] = sum_e emb[v,e] * conv_w[h,e,k]
    U = (emb_table @ conv_w.transpose(1, 0, 2).reshape(E, H * K))
    U = U.reshape(V, H, K).transpose(0, 2, 1).copy()       # [V, K, H]
    U[:, K - 1, :] += np.asarray(conv_b, np.float32)
    P1 = emb_table @ lin_w[:, :E].T + np.asarray(lin_b, np.float32)[None, :]
    W2T = lin_w[:, E:].T.copy()                            # [H, V]

    # [128, k, hc, vh, 128]: u_p[p, k, hc, vh, m] = U[vh*128+p, k, hc*128+m]
    u_p = (U.reshape(2, 128, K, H8, 128)
           .transpose(1, 2, 3, 0, 4)).astype(F8)
    w2_p = W2T.reshape(H8, 128, V).transpose(1, 0, 2).astype(F16)
    return np.ascontiguousarray(u_p), np.ascontiguousarray(w2_p), P1


def _onehot(tokens):
    """[128, 2, B, SPAD] fp8, left-padded with 15 zero columns per sequence."""
    tok = np.asarray(tokens).astype(np.int64)
    oh = np.zeros((128, 2, B, SPAD), F8)
    t = tok.ravel()
    b_idx = np.repeat(np.arange(B), S)
    col = np.tile(np.arange(S), B) + PAD
    oh[t % 128, t // 128, b_idx, col] = 1
    return oh


def kernel(input_sequence, emb_table, conv_w, conv_b, lin_w, lin_b):
    global LAST_RESULT
    import os
    if not TRACE:
        # the container's antenv lacks the axon NTFF hook; make sure an
        # ambient BASS_TRACE can't route us into that import path
        os.environ["BASS_NEVER_TRACE"] = "1"
    else:
        os.environ.pop("BASS_NEVER_TRACE", None)
    from concourse.bass_utils import run_bass_kernel_spmd

    u_p, w2_p, P1 = _pack_tables(emb_table, conv_w, conv_b, lin_w, lin_b)
    oh_full = _onehot(input_sequence)
    # emb-side logits: gather P1 rows per token, packed per 512-token tile
    # as [tile, p, m, v] with token row = tile*512 + m*128 + p
    tok = np.asarray(input_sequence).astype(np.int64)
    pe_all = P1[tok].astype(np.float32)                      # [B, S, V]
    pe_all = (pe_all.reshape(B * S // 512, 4, 128, V)
              .transpose(0, 2, 1, 3))                  # [tiles, 128, 4, V]

    ntt_core = SEQ_PER_CORE * NTT
    in_maps = []
    for c in range(NCORES):
        in_maps.append({
            "oh": np.ascontiguousarray(
                oh_full[:, :, c * SEQ_PER_CORE:(c + 1) * SEQ_PER_CORE, :]),
            "u": u_p, "w2": w2_p,
            "pe": np.ascontiguousarray(
                pe_all[c * ntt_core:(c + 1) * ntt_core]),
        })

    nc = _get_nc()
    res = run_bass_kernel_spmd(nc, in_maps, core_ids=list(range(NCORES)),
                               trace=TRACE)
    LAST_RESULT = res
    outs = [res.results[c]["out"] for c in range(NCORES)]   # [8192, 256] each
    full = np.concatenate(outs, axis=0).reshape(B, S, V)
    return np.ascontiguousarray(full.astype(np.float32))
